# revision 17
# baseline (speedup 1.0000x reference)
"""Multi-head attention (B=4, S=2048, E=1024, H=16, D=64) on 8 Trainium2 cores.

Sharding: core c handles batch b=c//2 and head-group g=c%2 (8 of 16 heads).

v3 design notes:
- All matmuls are bf16 inputs with f32 PSUM accumulation and run in the
  single (128,128) PE tile mode (no mode-switch drains, FWL-eligible):
  Q/K are stored zero-PADDED to 128 partitions per head (data in
  partitions 0-63, zeros in 64-127) so scores contract over 128.
- Phase C processes one (head, 1024-query-half) at a time: 16 key-tiles,
  scores -> exp (ACT, [128,1024] granularity) -> PV accumulated into a
  [65,1024] PSUM tile (V carries a ones column => softmax denominators
  in row 64). psS/psAt both double-buffered: 8 PSUM banks exactly.
- Softmax normalization off the critical path: reciprocal on DVE, DRAM
  bounce broadcast to 64 partitions, multiply into attnT (bf16).
- Phase D: out^T = sum_u w_out[u]^T @ attnT[u], f32 out, host adds pairs
  of cores, transposes, adds bias.
"""
import os
import sys

sys.path.insert(0, "/opt/trn_rl_repo")

import numpy as np
import ml_dtypes

import concourse.bass as bass
import concourse.mybir as mybir
import concourse.tile as tile
from concourse import bacc
from concourse.bass_utils import run_bass_kernel_spmd

B, S, E, H, D = 4, 2048, 1024, 16, 64
HPC = 8            # heads per core
NCORES = 8
P = 128
NKT = S // P       # 16 key tiles
W = 1024           # query-half width in phase C
f32 = mybir.dt.float32
bf16 = mybir.dt.bfloat16
AF = mybir.ActivationFunctionType
SCALE = 1.0 / 8.0  # 1/sqrt(D)
bfnp = ml_dtypes.bfloat16

_BUILD_CACHE = {}
LAST_RESULTS = None


def build_nc(s=S, repeat=1):
    nsq = s // 512        # 512-wide s chunks in phase A/B
    nst = s // P          # 128-wide s tiles
    nc = bacc.Bacc("TRN2", target_bir_lowering=False, debug=False,
                   num_devices=NCORES)

    xT = nc.dram_tensor("xT", [E, s], bf16, kind="ExternalInput").ap()
    w_qk = nc.dram_tensor("w_qk", [E, HPC * P], bf16, kind="ExternalInput").ap()
    b_qk = nc.dram_tensor("b_qk", [P, HPC], f32, kind="ExternalInput").ap()
    w_v = nc.dram_tensor("w_v", [E, HPC * D], bf16, kind="ExternalInput").ap()
    b_v = nc.dram_tensor("b_v", [P, HPC * D], f32, kind="ExternalInput").ap()
    w_out = nc.dram_tensor("w_out", [HPC * D, E], bf16,
                           kind="ExternalInput").ap()
    outT = nc.dram_tensor("outT", [E, s], bf16, kind="ExternalOutput").ap()
    scratch = nc.dram_tensor("scratch", [HPC * 2, W], f32).ap()  # recip bounce

    xT_r = xT.rearrange("(ko p) s -> p ko s", p=P)        # [128, 8, s]
    wqk_r = w_qk.rearrange("(ko p) f -> p ko f", p=P)     # [128, 8, 1024]
    wv_r = w_v.rearrange("(ko p) f -> p ko f", p=P)       # [128, 8, 512]
    wo_r = w_out.rearrange("(j p) f -> p j f", p=P)       # [128, 4, 1024]
    outT_r = outT.rearrange("(m p) s -> p m s", p=P)      # [128, 8, s]

    with tile.TileContext(nc) as tc:
        def body():
            from contextlib import ExitStack
            with ExitStack() as outer:
                persist = outer.enter_context(tc.tile_pool(name="persist",
                                                           bufs=1))
                qTp = persist.tile([P, HPC, s], bf16)   # padded Q^T per head
                kTp = persist.tile([P, HPC, s], bf16)   # padded K^T per head
                v_sb = persist.tile([P, nst, HPC, D + 1], bf16)  # V' ones col
                attnT = persist.tile([P, HPC // 2, s], bf16)
                bqk_sb = persist.tile([P, HPC], f32)
                bv_sb = persist.tile([P, HPC, D], f32)

                # gpsimd queue order: small bias loads + V weights first (A/B
                # needs them early), then the big zero-pad memsets (only
                # needed by phase C)
                nc.gpsimd.dma_start(bqk_sb[:], b_qk)
                nc.gpsimd.dma_start(
                    bv_sb[:], b_v.rearrange("p (h d) -> p h d", d=D))

                # psD lives below the A/B pools on the PSUM stack: its two
                # banks host the C-prefix (first head's scores+exp) during
                # the last A/B chunk, then phase-D partials during C.
                psD = outer.enter_context(
                    tc.tile_pool(name="psD", bufs=2, space="PSUM"))
                ppool = outer.enter_context(tc.tile_pool(name="pT", bufs=4))
                prefix_p = {}
                NPREF = 12

                # ---- Phase A/B: projections, streaming x^T in 512-col chunks
                with ExitStack() as ab:
                    xpool = ab.enter_context(tc.tile_pool(name="x", bufs=2))
                    wqk_pool = ab.enter_context(tc.tile_pool(name="wqk",
                                                             bufs=1))
                    wv_pool = ab.enter_context(tc.tile_pool(name="wv", bufs=1))
                    ktp_pool = ab.enter_context(tc.tile_pool(name="ktmp",
                                                             bufs=3))
                    psA = ab.enter_context(
                        tc.tile_pool(name="psA", bufs=3, space="PSUM"))
                    psB = ab.enter_context(
                        tc.tile_pool(name="psB", bufs=2, space="PSUM"))
                    wqk_sb = wqk_pool.tile([P, 8, HPC * P], bf16)
                    wv_sb = wv_pool.tile([P, 8, HPC * D], bf16)
                    # startup ordering: first x chunk and the first half of
                    # w_qk land before anything else so PE starts early
                    xt0 = xpool.tile([P, 8, 512], bf16)
                    nc.sync.dma_start(xt0[:], xT_r[:, :, 0:512])
                    nc.sync.dma_start(wqk_sb[:, 0:2], wqk_r[:, 0:2])
                    nc.sync.dma_start(wqk_sb[:, 2:5], wqk_r[:, 2:5])
                    nc.sync.dma_start(wqk_sb[:, 5:8], wqk_r[:, 5:8])
                    nc.gpsimd.dma_start(wv_sb[:], wv_r)
                    nc.gpsimd.memset(qTp[64:128], 0.0)
                    nc.gpsimd.memset(kTp[64:128], 0.0)
                    nc.gpsimd.memset(v_sb[:, :, :, D:D + 1], 1.0)

                    for q in range(nsq):
                        sq = slice(q * 512, (q + 1) * 512)
                        if q == nsq - 1:
                            # C-prefix: head 0's first-12-key-tile scores+exp
                            # run on ACT during this last A/B chunk, using the
                            # psD banks (idle until mid-C)
                            for kt in range(NPREF):
                                hv = []
                                for hf in range(2):
                                    pd = psD.tile([P, 512], f32,
                                                  name="pd", tag="pd")
                                    nc.tensor.matmul(
                                        pd[:],
                                        lhsT=kTp[:, 0, kt * P:(kt + 1) * P],
                                        rhs=qTp[:, 0, hf * 512:(hf + 1) * 512],
                                        start=True, stop=True)
                                    pTh = ppool.tile([P, 512], bf16,
                                                     name="pTh", tag="pTh",
                                                     bufs=2 * NPREF)
                                    nc.scalar.activation(pTh[:], pd[:],
                                                         AF.Exp, scale=SCALE)
                                    hv.append(pTh)
                                prefix_p[kt] = hv
                        if q == 0:
                            xt = xt0
                        else:
                            xt = xpool.tile([P, 8, 512], bf16)
                            nc.sync.dma_start(xt[:], xT_r[:, :, sq])
                        for m in range(HPC):  # per-head [q|k] feature tiles
                            ps = psA.tile([P, 512], f32)
                            for k in range(8):
                                nc.tensor.matmul(
                                    ps[:], lhsT=wqk_sb[:, k, m * P:(m + 1) * P],
                                    rhs=xt[:, k, :],
                                    start=(k == 0), stop=(k == 7))
                            # q features land on partitions 0:64 directly
                            nc.vector.tensor_scalar_add(
                                qTp[0:64, m, sq], ps[0:64],
                                bqk_sb[0:64, m:m + 1])
                            # k features are on partitions 64:128; bounce via
                            # SBUF->SBUF DMA down to partitions 0:64
                            kt_t = ktp_pool.tile([P, 512], bf16)
                            nc.vector.tensor_scalar_add(
                                kt_t[64:128, :], ps[64:128],
                                bqk_sb[64:128, m:m + 1])
                            nc.gpsimd.dma_start(kTp[0:64, m, sq],
                                                kt_t[64:128, :])
                        for stl in range(4):  # V: s tiles of 128 in this chunk
                            st = q * 4 + stl
                            ps = psB.tile([P, 512], f32)
                            for k in range(8):
                                nc.tensor.matmul(
                                    ps[:], lhsT=xt[:, k, stl * P:(stl + 1) * P],
                                    rhs=wv_sb[:, k, :],
                                    start=(k == 0), stop=(k == 7))
                            nc.vector.tensor_add(
                                v_sb[:, st, :, 0:D],
                                ps.rearrange("p (h d) -> p h d", d=D),
                                bv_sb[:])

                # ---- Phase C: attention, one (head, query-half) at a time
                cd = outer.enter_context(ExitStack())
                wo_pool = cd.enter_context(tc.tile_pool(name="wo", bufs=1))
                wo_sb = wo_pool.tile([P, HPC * D // P, E], bf16)
                nc.sync.dma_start(wo_sb[:], wo_r)

                with ExitStack() as c:
                    psS = c.enter_context(
                        tc.tile_pool(name="psS", bufs=2, space="PSUM"))
                    psAt = c.enter_context(
                        tc.tile_pool(name="psAt", bufs=1, space="PSUM"))
                    npool = c.enter_context(tc.tile_pool(name="norm", bufs=2))
                    apool = c.enter_context(tc.tile_pool(name="att", bufs=2))
                    spool = c.enter_context(tc.tile_pool(name="stage", bufs=2))
                    oaccp = c.enter_context(tc.tile_pool(name="oacc", bufs=1))
                    opool = c.enter_context(tc.tile_pool(name="oout", bufs=3))
                    o_acc = oaccp.tile([P, 8, s], bf16)

                    # Phase-D work items (one matmul + one DVE accumulate),
                    # interleaved one-per-kt into the following pair's
                    # ACT-bound attention loop.
                    pending_d = []

                    def emit_d():
                        u, m, c4 = pending_d.pop(0)
                        cs = slice(c4 * 512, (c4 + 1) * 512)
                        pd = psD.tile([P, 512], f32, name="pd", tag="pd")
                        nc.tensor.matmul(
                            pd[:], lhsT=wo_sb[:, u, m * P:(m + 1) * P],
                            rhs=attnT[:, u, cs], start=True, stop=True)
                        if u == 0:
                            nc.vector.tensor_copy(o_acc[:, m, cs], pd[:])
                        elif u < HPC // 2 - 1:
                            nc.vector.tensor_add(o_acc[:, m, cs], pd[:],
                                                 o_acc[:, m, cs])
                        else:
                            oo = opool.tile([P, 512], bf16,
                                            name="oo", tag="oo")
                            nc.vector.tensor_add(oo[:], pd[:],
                                                 o_acc[:, m, cs])
                            nc.sync.dma_start(outT_r[:, m, cs], oo[:])

                    for h in range(HPC):
                        for q2 in range(s // W):
                            qs = slice(q2 * W, (q2 + 1) * W)
                            at = psAt.tile([D + 1, W], f32,
                                           name="at", tag="at")

                            def emit_pv(kt, pT):
                                for hf in range(2):
                                    fs = slice(hf * 512, (hf + 1) * 512)
                                    rhs = (pT[hf][:, :] if isinstance(pT, list)
                                           else pT[:, fs])
                                    nc.tensor.matmul(
                                        at[:, fs],
                                        lhsT=v_sb[:, kt, h, :],
                                        rhs=rhs,
                                        start=(kt == 0), stop=(kt == NKT - 1))

                            prev = None
                            for kt in range(NKT):
                                if h == 0 and q2 == 0 and kt < NPREF:
                                    pT = prefix_p[kt]  # precomputed in A/B
                                else:
                                    ps = psS.tile([P, W], f32,
                                                  name="ps_s", tag="ps_s")
                                    for hf in range(2):
                                        cs = slice(q2 * W + hf * 512,
                                                   q2 * W + (hf + 1) * 512)
                                        nc.tensor.matmul(
                                            ps[:, hf * 512:(hf + 1) * 512],
                                            lhsT=kTp[:, h, kt * P:(kt + 1) * P],
                                            rhs=qTp[:, h, cs],
                                            start=True, stop=True)
                                    pT = ppool.tile([P, W], bf16,
                                                    name="pT", tag="pT")
                                    nc.scalar.activation(pT[:], ps[:], AF.Exp,
                                                         scale=SCALE)
                                if prev is not None:
                                    emit_pv(*prev)
                                prev = (kt, pT)
                                if pending_d and 3 <= kt <= NKT - 4:
                                    emit_d()
                            emit_pv(*prev)

                            # evacuate accumulator (frees the single PSUM
                            # at-buffer fast), then normalize off PE path
                            att = apool.tile([D + 1, W], f32,
                                             name="att", tag="att")
                            nc.vector.tensor_copy(att[:], at[:])
                            idx = h * 2 + q2
                            rec = npool.tile([1, W], f32, tag="rec")
                            nc.vector.reciprocal(rec[:], att[D:D + 1, :])
                            nc.gpsimd.dma_start(scratch[idx:idx + 1, :],
                                                rec[:])
                            bcst = npool.tile([64, W], f32, tag="bc")
                            nc.gpsimd.dma_start(
                                bcst[:],
                                scratch[idx:idx + 1, :].partition_broadcast(64)
                                .rearrange("p one w -> p (one w)"))
                            u = h // 2
                            if h % 2 == 0:
                                nc.vector.tensor_mul(
                                    attnT[0:64, u, qs], att[0:D, :], bcst[:])
                            else:
                                stg = spool.tile([64, W], bf16, tag="stg")
                                nc.vector.tensor_mul(
                                    stg[:], att[0:D, :], bcst[:])
                                nc.sync.dma_start(attnT[64:128, u, qs],
                                                  stg[:])
                            if h % 2 == 1:
                                # this query-half of pair u is complete on
                                # both heads: its D chunks can run now
                                pending_d.extend(
                                    (u, m, 2 * q2 + cc)
                                    for m in range(8) for cc in range(2))
                    while pending_d:
                        emit_d()

        if repeat > 1:
            with tc.For_i(0, repeat, 1):
                body()
        else:
            body()

    nc.compile()
    return nc


def _get_nc(s=S, repeat=1):
    key = (s, repeat)
    if key not in _BUILD_CACHE:
        _BUILD_CACHE[key] = build_nc(s=s, repeat=repeat)
    return _BUILD_CACHE[key]


def shard_inputs(x, w_qkv, b_qkv, w_out, b_out):
    """Host-side sharding: per-core input maps."""
    in_maps = []
    for c in range(NCORES):
        b, g = c // 2, c % 2
        heads = [g * HPC + i for i in range(HPC)]
        # w_qk block h = [wq_h (E x 64) | wk_h (E x 64)]
        qk_cols, qk_bias = [], []
        for hh in heads:
            qk_cols.append(w_qkv[:, hh * 192:hh * 192 + 128])
            qk_bias.append(b_qkv[hh * 192:hh * 192 + 128])
        w_qk_c = np.concatenate(qk_cols, axis=1).astype(bfnp)
        # bias [128, HPC]: col m = [bq_h(64); bk_h(64)]
        b_qk_c = np.stack(qk_bias, axis=1).astype(np.float32)
        b_qk_c = np.ascontiguousarray(b_qk_c)
        w_v_c = np.concatenate(
            [w_qkv[:, hh * 192 + 128:hh * 192 + 192] for hh in heads],
            axis=1).astype(bfnp)
        b_v_c = np.ascontiguousarray(np.broadcast_to(np.concatenate(
            [b_qkv[hh * 192 + 128:hh * 192 + 192] for hh in heads])[None, :],
            (P, HPC * D)).astype(np.float32))
        w_out_c = np.concatenate(
            [w_out[hh * D:(hh + 1) * D, :] for hh in heads],
            axis=0).astype(bfnp)
        xT_c = np.ascontiguousarray(x[b].T).astype(bfnp)
        in_maps.append({
            "xT": xT_c, "w_qk": np.ascontiguousarray(w_qk_c),
            "b_qk": b_qk_c,
            "w_v": np.ascontiguousarray(w_v_c), "b_v": b_v_c,
            "w_out": np.ascontiguousarray(w_out_c),
        })
    return in_maps


def unshard_output(results, b_out):
    out = np.empty((B, S, E), dtype=np.float32)
    for b in range(B):
        acc = (np.asarray(results[2 * b]["outT"], dtype=np.float32)
               + np.asarray(results[2 * b + 1]["outT"], dtype=np.float32))
        out[b] = acc.T + b_out
    return out


def kernel(x, w_qkv, b_qkv, w_out, b_out):
    global LAST_RESULTS
    x = np.asarray(x, dtype=np.float32)
    w_qkv = np.asarray(w_qkv, dtype=np.float32)
    b_qkv = np.asarray(b_qkv, dtype=np.float32)
    w_out = np.asarray(w_out, dtype=np.float32)
    b_out = np.asarray(b_out, dtype=np.float32)

    nc = _get_nc()
    in_maps = shard_inputs(x, w_qkv, b_qkv, w_out, b_out)
    try:
        res = run_bass_kernel_spmd(nc, in_maps, list(range(NCORES)))
    except ModuleNotFoundError:
        os.environ["BASS_NEVER_TRACE"] = "1"
        res = run_bass_kernel_spmd(nc, in_maps, list(range(NCORES)))
    LAST_RESULTS = res
    return unshard_output(res.results, b_out)


# revision 22
# speedup vs baseline: 1.1127x; 1.1127x over previous
"""Multi-head attention (B=4, S=2048, E=1024, H=16, D=64) on 8 Trainium2 cores.

Sharding: core c handles batch b=c//2 and head-group g=c%2 (8 of 16 heads).

v3 design notes:
- All matmuls are bf16 inputs with f32 PSUM accumulation and run in the
  single (128,128) PE tile mode (no mode-switch drains, FWL-eligible):
  Q/K are stored zero-PADDED to 128 partitions per head (data in
  partitions 0-63, zeros in 64-127) so scores contract over 128.
- Phase C processes one (head, 1024-query-half) at a time: 16 key-tiles,
  scores -> exp (ACT, [128,1024] granularity) -> PV accumulated into a
  [65,1024] PSUM tile (V carries a ones column => softmax denominators
  in row 64). psS/psAt both double-buffered: 8 PSUM banks exactly.
- Softmax normalization off the critical path: reciprocal on DVE, DRAM
  bounce broadcast to 64 partitions, multiply into attnT (bf16).
- Phase D: out^T = sum_u w_out[u]^T @ attnT[u], f32 out, host adds pairs
  of cores, transposes, adds bias.
"""
import os
import sys

sys.path.insert(0, "/opt/trn_rl_repo")

import numpy as np
import ml_dtypes

import concourse.bass as bass
import concourse.mybir as mybir
import concourse.tile as tile
from concourse import bacc
from concourse.bass_utils import run_bass_kernel_spmd

B, S, E, H, D = 4, 2048, 1024, 16, 64
HPC = 8            # heads per core
NCORES = 8
P = 128
NKT = S // P       # 16 key tiles
W = 1024           # query-half width in phase C
f32 = mybir.dt.float32
bf16 = mybir.dt.bfloat16
AF = mybir.ActivationFunctionType
SCALE = 1.0 / 8.0  # 1/sqrt(D)
bfnp = ml_dtypes.bfloat16

_BUILD_CACHE = {}
LAST_RESULTS = None


def build_nc(s=S, repeat=1):
    nsq = s // 512        # 512-wide s chunks in phase A/B
    nst = s // P          # 128-wide s tiles
    nc = bacc.Bacc("TRN2", target_bir_lowering=False, debug=False,
                   num_devices=NCORES)

    xT = nc.dram_tensor("xT", [E, s], bf16, kind="ExternalInput").ap()
    w_qk = nc.dram_tensor("w_qk", [E, HPC * P], bf16, kind="ExternalInput").ap()
    b_qk = nc.dram_tensor("b_qk", [P, HPC], f32, kind="ExternalInput").ap()
    w_v = nc.dram_tensor("w_v", [E, HPC * D], bf16, kind="ExternalInput").ap()
    b_v = nc.dram_tensor("b_v", [P, HPC * D], f32, kind="ExternalInput").ap()
    w_out = nc.dram_tensor("w_out", [HPC * D, E], bf16,
                           kind="ExternalInput").ap()
    outT = nc.dram_tensor("outT", [E, s], bf16, kind="ExternalOutput").ap()
    scratch = nc.dram_tensor("scratch", [HPC * 2, W], f32).ap()  # recip bounce

    xT_r = xT.rearrange("(ko p) s -> p ko s", p=P)        # [128, 8, s]
    wqk_r = w_qk.rearrange("(ko p) f -> p ko f", p=P)     # [128, 8, 1024]
    wv_r = w_v.rearrange("(ko p) f -> p ko f", p=P)       # [128, 8, 512]
    wo_r = w_out.rearrange("(j p) f -> p j f", p=P)       # [128, 4, 1024]
    outT_r = outT.rearrange("(m p) s -> p m s", p=P)      # [128, 8, s]

    with tile.TileContext(nc) as tc:
        def body():
            from contextlib import ExitStack
            with ExitStack() as outer:
                persist = outer.enter_context(tc.tile_pool(name="persist",
                                                           bufs=1))
                qTp = persist.tile([P, HPC, s], bf16)   # padded Q^T per head
                kTp = persist.tile([P, HPC, s], bf16)   # padded K^T per head
                v_sb = persist.tile([P, nst, HPC, D + 1], bf16)  # V' ones col
                attnT = persist.tile([P, HPC // 2, s], bf16)
                bqk_sb = persist.tile([P, HPC], f32)
                bv_sb = persist.tile([P, HPC, D], f32)

                # gpsimd queue order: small bias loads + V weights first (A/B
                # needs them early), then the big zero-pad memsets (only
                # needed by phase C)
                nc.gpsimd.dma_start(bqk_sb[:], b_qk)
                nc.gpsimd.dma_start(
                    bv_sb[:], b_v.rearrange("p (h d) -> p h d", d=D))

                ppool = outer.enter_context(tc.tile_pool(name="pT", bufs=4))
                prefix_p = {}
                NPREF = 12

                # ---- Phase A/B: projections, streaming x^T in 512-col chunks
                with ExitStack() as ab:
                    xpool = ab.enter_context(tc.tile_pool(name="x", bufs=2))
                    wqk_pool = ab.enter_context(tc.tile_pool(name="wqk",
                                                             bufs=1))
                    wv_pool = ab.enter_context(tc.tile_pool(name="wv", bufs=1))
                    ktp_pool = ab.enter_context(tc.tile_pool(name="ktmp",
                                                             bufs=3))
                    psA = ab.enter_context(
                        tc.tile_pool(name="psA", bufs=3, space="PSUM"))
                    psB = ab.enter_context(
                        tc.tile_pool(name="psB", bufs=2, space="PSUM"))
                    # prefix pool: 2 banks, freed when A/B closes (before the
                    # 8-bank C pools open)
                    psPre = ab.enter_context(
                        tc.tile_pool(name="psPre", bufs=2, space="PSUM"))
                    wqk_sb = wqk_pool.tile([P, 8, HPC * P], bf16)
                    wv_sb = wv_pool.tile([P, 8, HPC * D], bf16)
                    # startup ordering: first x chunk and the first half of
                    # w_qk land before anything else so PE starts early
                    xt0 = xpool.tile([P, 8, 512], bf16)
                    nc.sync.dma_start(xt0[:], xT_r[:, :, 0:512])
                    nc.sync.dma_start(wqk_sb[:, 0:2], wqk_r[:, 0:2])
                    nc.sync.dma_start(wqk_sb[:, 2:5], wqk_r[:, 2:5])
                    nc.sync.dma_start(wqk_sb[:, 5:8], wqk_r[:, 5:8])
                    nc.gpsimd.dma_start(wv_sb[:], wv_r)
                    nc.gpsimd.memset(qTp[64:128], 0.0)
                    nc.gpsimd.memset(kTp[64:128], 0.0)
                    nc.gpsimd.memset(v_sb[:, :, :, D:D + 1], 1.0)

                    for q in range(nsq):
                        sq = slice(q * 512, (q + 1) * 512)
                        if q == nsq - 1:
                            # C-prefix: head 0's first-12-key-tile scores+exp
                            # run on ACT during this last A/B chunk, using the
                            # psD banks (idle until mid-C)
                            for kt in range(NPREF):
                                hv = []
                                for hf in range(2):
                                    pd = psPre.tile([P, 512], f32,
                                                    name="pd", tag="pd")
                                    nc.tensor.matmul(
                                        pd[:],
                                        lhsT=kTp[:, 0, kt * P:(kt + 1) * P],
                                        rhs=qTp[:, 0, hf * 512:(hf + 1) * 512],
                                        start=True, stop=True)
                                    pTh = ppool.tile([P, 512], bf16,
                                                     name="pTh", tag="pTh",
                                                     bufs=2 * NPREF)
                                    nc.scalar.activation(pTh[:], pd[:],
                                                         AF.Exp, scale=SCALE)
                                    hv.append(pTh)
                                prefix_p[kt] = hv
                        if q == 0:
                            xt = xt0
                        else:
                            xt = xpool.tile([P, 8, 512], bf16)
                            nc.sync.dma_start(xt[:], xT_r[:, :, sq])
                        for m in range(HPC):  # per-head [q|k] feature tiles
                            ps = psA.tile([P, 512], f32)
                            for k in range(8):
                                nc.tensor.matmul(
                                    ps[:], lhsT=wqk_sb[:, k, m * P:(m + 1) * P],
                                    rhs=xt[:, k, :],
                                    start=(k == 0), stop=(k == 7))
                            # q features land on partitions 0:64 directly
                            nc.vector.tensor_scalar_add(
                                qTp[0:64, m, sq], ps[0:64],
                                bqk_sb[0:64, m:m + 1])
                            # k features are on partitions 64:128; bounce via
                            # SBUF->SBUF DMA down to partitions 0:64
                            kt_t = ktp_pool.tile([P, 512], bf16)
                            nc.vector.tensor_scalar_add(
                                kt_t[64:128, :], ps[64:128],
                                bqk_sb[64:128, m:m + 1])
                            nc.gpsimd.dma_start(kTp[0:64, m, sq],
                                                kt_t[64:128, :])
                        for stl in range(4):  # V: s tiles of 128 in this chunk
                            st = q * 4 + stl
                            ps = psB.tile([P, 512], f32)
                            for k in range(8):
                                nc.tensor.matmul(
                                    ps[:], lhsT=xt[:, k, stl * P:(stl + 1) * P],
                                    rhs=wv_sb[:, k, :],
                                    start=(k == 0), stop=(k == 7))
                            nc.vector.tensor_add(
                                v_sb[:, st, :, 0:D],
                                ps.rearrange("p (h d) -> p h d", d=D),
                                bv_sb[:])

                # ---- Phase C: attention, one (head, query-half) at a time
                cd = outer.enter_context(ExitStack())
                wo_pool = cd.enter_context(tc.tile_pool(name="wo", bufs=1))
                wo_sb = wo_pool.tile([P, HPC * D // P, E], bf16)
                nc.sync.dma_start(wo_sb[:], wo_r)

                with ExitStack() as c:
                    psS = c.enter_context(
                        tc.tile_pool(name="psS", bufs=2, space="PSUM"))
                    psAt = c.enter_context(
                        tc.tile_pool(name="psAt", bufs=2, space="PSUM"))
                    npool = c.enter_context(tc.tile_pool(name="norm", bufs=2))
                    spool = c.enter_context(tc.tile_pool(name="stage", bufs=2))
                    for h in range(HPC):
                        for q2 in range(s // W):
                            qs = slice(q2 * W, (q2 + 1) * W)
                            at = psAt.tile([D + 1, W], f32,
                                           name="at", tag="at")

                            def emit_pv(kt, pT):
                                for hf in range(2):
                                    fs = slice(hf * 512, (hf + 1) * 512)
                                    rhs = (pT[hf][:, :] if isinstance(pT, list)
                                           else pT[:, fs])
                                    nc.tensor.matmul(
                                        at[:, fs],
                                        lhsT=v_sb[:, kt, h, :],
                                        rhs=rhs,
                                        start=(kt == 0), stop=(kt == NKT - 1))

                            prev = None
                            for kt in range(NKT):
                                if h == 0 and q2 == 0 and kt < NPREF:
                                    pT = prefix_p[kt]  # precomputed in A/B
                                else:
                                    ps = psS.tile([P, W], f32,
                                                  name="ps_s", tag="ps_s")
                                    for hf in range(2):
                                        cs = slice(q2 * W + hf * 512,
                                                   q2 * W + (hf + 1) * 512)
                                        nc.tensor.matmul(
                                            ps[:, hf * 512:(hf + 1) * 512],
                                            lhsT=kTp[:, h, kt * P:(kt + 1) * P],
                                            rhs=qTp[:, h, cs],
                                            start=True, stop=True)
                                    pT = ppool.tile([P, W], bf16,
                                                    name="pT", tag="pT")
                                    nc.scalar.activation(pT[:], ps[:], AF.Exp,
                                                         scale=SCALE)
                                if prev is not None:
                                    emit_pv(*prev)
                                prev = (kt, pT)
                            emit_pv(*prev)

                            # normalization (off PE critical path)
                            idx = h * 2 + q2
                            rec = npool.tile([1, W], f32, tag="rec")
                            nc.vector.reciprocal(rec[:], at[D:D + 1, :])
                            nc.gpsimd.dma_start(scratch[idx:idx + 1, :],
                                                rec[:])
                            bcst = npool.tile([64, W], f32, tag="bc")
                            nc.gpsimd.dma_start(
                                bcst[:],
                                scratch[idx:idx + 1, :].partition_broadcast(64)
                                .rearrange("p one w -> p (one w)"))
                            u = h // 2
                            if h % 2 == 0:
                                nc.vector.tensor_mul(
                                    attnT[0:64, u, qs], at[0:D, :], bcst[:])
                            else:
                                stg = spool.tile([64, W], bf16, tag="stg")
                                nc.vector.tensor_mul(
                                    stg[:], at[0:D, :], bcst[:])
                                nc.sync.dma_start(attnT[64:128, u, qs],
                                                  stg[:])

                # ---- Phase D: output projection out^T = sum_u wo_u^T@attnT_u
                with ExitStack() as d:
                    psD = d.enter_context(
                        tc.tile_pool(name="psD", bufs=2, space="PSUM"))
                    opool = d.enter_context(tc.tile_pool(name="osb", bufs=2))
                    for m in range(8):
                        ps_o = psD.tile([P, s], f32)
                        for u in range(HPC // 2):
                            for n4 in range(nsq):
                                nc.tensor.matmul(
                                    ps_o[:, n4 * 512:(n4 + 1) * 512],
                                    lhsT=wo_sb[:, u, m * P:(m + 1) * P],
                                    rhs=attnT[:, u, n4 * 512:(n4 + 1) * 512],
                                    start=(u == 0), stop=(u == HPC // 2 - 1))
                        o_sb = opool.tile([P, s], bf16)
                        nc.vector.tensor_copy(o_sb[:], ps_o[:])
                        nc.sync.dma_start(outT_r[:, m, :], o_sb[:])

        if repeat > 1:
            with tc.For_i(0, repeat, 1):
                body()
        else:
            body()

    nc.compile()
    return nc


def _get_nc(s=S, repeat=1):
    key = (s, repeat)
    if key not in _BUILD_CACHE:
        _BUILD_CACHE[key] = build_nc(s=s, repeat=repeat)
    return _BUILD_CACHE[key]


def shard_inputs(x, w_qkv, b_qkv, w_out, b_out):
    """Host-side sharding: per-core input maps."""
    in_maps = []
    for c in range(NCORES):
        b, g = c // 2, c % 2
        heads = [g * HPC + i for i in range(HPC)]
        # w_qk block h = [wq_h (E x 64) | wk_h (E x 64)]
        qk_cols, qk_bias = [], []
        for hh in heads:
            qk_cols.append(w_qkv[:, hh * 192:hh * 192 + 128])
            qk_bias.append(b_qkv[hh * 192:hh * 192 + 128])
        w_qk_c = np.concatenate(qk_cols, axis=1).astype(bfnp)
        # bias [128, HPC]: col m = [bq_h(64); bk_h(64)]
        b_qk_c = np.stack(qk_bias, axis=1).astype(np.float32)
        b_qk_c = np.ascontiguousarray(b_qk_c)
        w_v_c = np.concatenate(
            [w_qkv[:, hh * 192 + 128:hh * 192 + 192] for hh in heads],
            axis=1).astype(bfnp)
        b_v_c = np.ascontiguousarray(np.broadcast_to(np.concatenate(
            [b_qkv[hh * 192 + 128:hh * 192 + 192] for hh in heads])[None, :],
            (P, HPC * D)).astype(np.float32))
        w_out_c = np.concatenate(
            [w_out[hh * D:(hh + 1) * D, :] for hh in heads],
            axis=0).astype(bfnp)
        xT_c = np.ascontiguousarray(x[b].T).astype(bfnp)
        in_maps.append({
            "xT": xT_c, "w_qk": np.ascontiguousarray(w_qk_c),
            "b_qk": b_qk_c,
            "w_v": np.ascontiguousarray(w_v_c), "b_v": b_v_c,
            "w_out": np.ascontiguousarray(w_out_c),
        })
    return in_maps


def unshard_output(results, b_out):
    out = np.empty((B, S, E), dtype=np.float32)
    for b in range(B):
        acc = (np.asarray(results[2 * b]["outT"], dtype=np.float32)
               + np.asarray(results[2 * b + 1]["outT"], dtype=np.float32))
        out[b] = acc.T + b_out
    return out


def kernel(x, w_qkv, b_qkv, w_out, b_out):
    global LAST_RESULTS
    x = np.asarray(x, dtype=np.float32)
    w_qkv = np.asarray(w_qkv, dtype=np.float32)
    b_qkv = np.asarray(b_qkv, dtype=np.float32)
    w_out = np.asarray(w_out, dtype=np.float32)
    b_out = np.asarray(b_out, dtype=np.float32)

    nc = _get_nc()
    in_maps = shard_inputs(x, w_qkv, b_qkv, w_out, b_out)
    try:
        res = run_bass_kernel_spmd(nc, in_maps, list(range(NCORES)))
    except ModuleNotFoundError:
        os.environ["BASS_NEVER_TRACE"] = "1"
        res = run_bass_kernel_spmd(nc, in_maps, list(range(NCORES)))
    LAST_RESULTS = res
    return unshard_output(res.results, b_out)
